# revision 17
# baseline (speedup 1.0000x reference)
"""AdaptiveKNN Trainium2 kernel (8 NeuronCores, SPMD).

Sharding: data-parallel over batch B=2 across pairs-of-4 cores; within a
batch, the N=8192 query rows are row-sharded 4 ways (2048 rows/core).
Each core computes its [2048, 8192] distance block, per-row top-128
neighbors (sorted, with indices), the radius count, and (after a tiny
AllReduce for the batch mean density) the adaptive-k mask.

Distance values are computed with the exact same f32 rounding structure
as the reference (fp32 PE matmul for the dot product, single-rounding
broadcast add for sq_i+sq_j, fused 2*g - ssum with exact *2) so selected
values/order match a neuron-run reference bitwise.

Selection per 128-query tile:
  1) per-segment (S=128 -> 64 segs) top-8 values+local indices (DVE max8/max_index)
  2) cheap per-row threshold tau = 34th largest of the seg-4th-bests
     (guarantees >= 136 elements above tau)
  3) compact all elements >= tau (plus filler) to C=352 slots via
     rank-from-prefix-scan + GPSIMD local_scatter (16-bit pair trick for f32)
  4) exact 17x8 max8/max_index/match_replace rounds on the compacted array
  5) invert the selection permutation with two more local_scatters to get
     original indices; remove the self-match by index value; emit top-128.
"""

import os
import sys

sys.path.insert(0, "/opt/trn_rl_repo")

import numpy as np

import concourse.bass as bass
import concourse.bacc as bacc
import concourse.mybir as mybir
from concourse.tile import TileContext
from concourse.bass_utils import run_bass_kernel_spmd

F32 = mybir.dt.float32
U16 = mybir.dt.uint16
I16 = mybir.dt.int16
I32 = mybir.dt.int32
U8 = mybir.dt.uint8
AF = mybir.ActivationFunctionType
OP = mybir.AluOpType

B, N, D = 2, 8192, 64
K_BASE, K_MIN, K_MAX = 32, 8, 128
RADIUS, EPS = 0.05, 1e-8
KOUT = 128

P = 128            # partitions / queries per tile
RPC = 2048         # rows per core
NT = RPC // P      # row tiles per core (16)
NBLK = N // 512    # 512-wide candidate blocks per tile (16)
SEG = 64           # segment size for first-level top-8
NSEG = N // SEG    # 128
POOL = NSEG * 8    # 1024
CCAP = 352         # compaction capacity (empirical max 319 + margin)
ROUNDS = 17        # 17*8 = 136 extracted (incl self)
SEL = ROUNDS * 8   # 136
NEG = -3.0e38
D32 = float(np.float32(N * RADIUS**3 + EPS))  # density denominator, f32
INVD = float(np.float32(1.0 / np.float64(np.float32(D32))))  # XLA-folded 1/D


def build_program(sim_single_core=False):
    nc = bacc.Bacc("TRN2", target_bir_lowering=False,
                   num_devices=1 if sim_single_core else 8)

    ptsm = nc.declare_dram_parameter("ptsm", [3, N], F32, isOutput=False)
    ptss = nc.declare_dram_parameter("ptss", [3, RPC], F32, isOutput=False)
    sqq = nc.declare_dram_parameter("sqq", [P, NT], F32, isOutput=False)
    sqc = nc.declare_dram_parameter("sqc", [1, N], F32, isOutput=False)
    selfidx = nc.declare_dram_parameter("selfidx", [P, NT], F32, isOutput=False)
    bsel = nc.declare_dram_parameter("bsel", [1, 2], F32, isOutput=False)
    gam = nc.declare_dram_parameter("gamma_param", [1, 1], F32, isOutput=False)
    oidx = nc.declare_dram_parameter("oidx", [RPC, KOUT], I32, isOutput=True)
    odist = nc.declare_dram_parameter("odist", [RPC, KOUT], F32, isOutput=True)
    omask = nc.declare_dram_parameter("omask", [RPC, KOUT], U8, isOutput=True)

    with TileContext(nc) as tc:
        with (
            tc.tile_pool(name="const", bufs=1) as cpool,
            tc.tile_pool(name="sblk", bufs=3) as spool,
            tc.tile_pool(name="blk", bufs=2) as bpool,
            tc.tile_pool(name="poolp", bufs=2) as ppool,
            tc.tile_pool(name="scratch", bufs=1) as scr,
            tc.tile_pool(name="outs", bufs=2) as opool,
            tc.tile_pool(name="psum", bufs=6, space="PSUM") as psum,
            tc.tile_pool(name="dram", bufs=1, space="DRAM") as dpool,
        ):
            # ---- persistent constants / inputs ----
            ptsm_sb = cpool.tile([3, N], F32)
            ptss_sb = cpool.tile([3, RPC], F32)
            sqq_sb = cpool.tile([P, NT], F32)
            sqc_sb = cpool.tile([1, N], F32)
            selfidx_sb = cpool.tile([P, NT], F32)
            bsel_sb = cpool.tile([1, 2], F32)
            gam_sb = cpool.tile([1, 1], F32)
            nc.sync.dma_start(out=ptsm_sb[:], in_=ptsm[:])
            nc.sync.dma_start(out=ptss_sb[:], in_=ptss[:])
            nc.sync.dma_start(out=sqq_sb[:], in_=sqq[:])
            nc.sync.dma_start(out=sqc_sb[:], in_=sqc[:])
            nc.sync.dma_start(out=selfidx_sb[:], in_=selfidx[:])
            nc.sync.dma_start(out=bsel_sb[:], in_=bsel[:])
            nc.sync.dma_start(out=gam_sb[:], in_=gam[:])

            sqcb = cpool.tile([P, N], F32)
            nc.gpsimd.partition_broadcast(sqcb[:], sqc_sb[:])

            # constants built on device
            segbase = cpool.tile([P, POOL], U16)       # (q>>3)*128
            nc.gpsimd.iota(segbase[:], pattern=[[SEG, NSEG], [0, 8]], base=0,
                           channel_multiplier=0)
            iota1u = cpool.tile([P, POOL], U16)        # 1..512
            nc.gpsimd.iota(iota1u[:], pattern=[[1, POOL]], base=1,
                           channel_multiplier=0)
            iota1f = cpool.tile([P, POOL], F32)
            nc.vector.tensor_copy(iota1f[:], iota1u[:])
            iota0f = cpool.tile([P, SEL], F32)         # 0..135
            nc.vector.tensor_scalar(iota0f[:], iota1f[:, :SEL], 1.0, None,
                                    op0=OP.subtract)
            zeros = cpool.tile([P, POOL], F32)
            nc.vector.memset(zeros[:], 0.0)
            epscol = cpool.tile([P, 1], F32)
            nc.vector.memset(epscol[:], EPS)
            # c + 0.5 grid: neuron's f32->int32 cast rounds to nearest, so
            # mask[c] = c < round(k) == (c + 0.5 <= k) (ties ~never hit)
            iotah = cpool.tile([P, KOUT], F32)
            nc.vector.tensor_scalar(iotah[:], iota1f[:, :KOUT], 0.5, None,
                                    op0=OP.subtract)

            cnts = cpool.tile([P, NT], F32)            # radius counts per row

            # ---- per row-tile main loop ----
            for t in range(NT):
                Pv = ppool.tile([P, POOL], F32, tag="Pv")
                PI = ppool.tile([P, POOL], U16, tag="PI")

                for j in range(NBLK):
                    g_ps = psum.tile([P, 512], F32, tag="g_ps")
                    nc.tensor.matmul(
                        g_ps[:],
                        ptss_sb[:, t * P:(t + 1) * P],
                        ptsm_sb[:, j * 512:(j + 1) * 512],
                        start=True, stop=True,
                    )
                    ssum = bpool.tile([P, 512], F32, tag="ssum")
                    nc.scalar.activation(ssum[:], sqcb[:, j * 512:(j + 1) * 512],
                                         AF.Identity, bias=sqq_sb[:, t:t + 1],
                                         scale=1.0)
                    sb = spool.tile([P, 512], F32, tag="sb")
                    nc.vector.scalar_tensor_tensor(
                        sb[:], g_ps[:], 2.0, ssum[:],
                        op0=OP.mult, op1=OP.subtract)
                    for sg in range(8):
                        seg = 8 * j + sg
                        sl = sb[:, sg * SEG:(sg + 1) * SEG]
                        nc.vector.max(out=Pv[:, seg * 8:(seg + 1) * 8], in_=sl)
                        nc.vector.max_index(PI[:, seg * 8:(seg + 1) * 8],
                                            Pv[:, seg * 8:(seg + 1) * 8], sl)

                # ---- tau = 46th largest of the seg 3rd-bests ----
                P3c = Pv.rearrange("p (s k) -> p s k", k=8)[:, :, 2]  # [P, NSEG]
                w4 = scr.tile([P, NSEG], F32, tag="w4")
                m8 = scr.tile([P, 8], F32, tag="m8")
                nc.vector.max(out=m8[:], in_=P3c)
                nc.vector.match_replace(w4[:], m8[:], P3c, NEG)
                for r in range(4):
                    nc.vector.max(out=m8[:], in_=w4[:])
                    nc.vector.match_replace(w4[:], m8[:], w4[:], NEG)
                m8b = scr.tile([P, 8], F32, tag="m8b")
                nc.vector.max(out=m8b[:], in_=w4[:])   # ranks 41..48
                tau = m8b[:, 5:6]                       # 46th largest

                # ---- compaction ranks ----
                msk = scr.tile([P, POOL], F32, tag="msk")
                nc.vector.tensor_scalar(msk[:], Pv[:], tau, None, op0=OP.is_ge)
                rank1 = scr.tile([P, POOL], F32, tag="rank1")
                nc.vector.tensor_tensor_scan(rank1[:], msk[:], zeros[:], 0.0,
                                             op0=OP.add, op1=OP.add)
                rank1u = scr.tile([P, POOL], F32, tag="wC")
                nc.vector.tensor_sub(rank1u[:], iota1f[:], rank1[:])
                posu = scr.tile([P, POOL], F32, tag="wD")
                nc.vector.tensor_scalar(posu[:], rank1u[:], rank1[:, POOL - 1:POOL],
                                        None, op0=OP.add)
                dru = scr.tile([P, POOL], F32, tag="wE")
                nc.vector.tensor_sub(dru[:], rank1[:], posu[:])
                drm = scr.tile([P, POOL], F32, tag="wC")
                nc.vector.tensor_mul(drm[:], msk[:], dru[:])
                pos1 = scr.tile([P, POOL], F32, tag="rank1")
                nc.vector.tensor_add(pos1[:], posu[:], drm[:])
                mask2 = scr.tile([P, POOL], F32, tag="msk")
                nc.vector.tensor_scalar(mask2[:], pos1[:], float(CCAP), None,
                                        op0=OP.is_le)
                posc = scr.tile([P, POOL], F32, tag="wE")
                nc.vector.tensor_mul(posc[:], pos1[:], mask2[:])
                idxB = scr.tile([P, POOL], I16, tag="idxB")
                nc.vector.tensor_scalar(idxB[:], posc[:], 1.0, None,
                                        op0=OP.subtract)
                idxpair = scr.tile([P, 2 * POOL], I16, tag="idxpair")
                ipv = idxpair.rearrange("p (a b) -> p a b", b=2)
                nc.vector.tensor_scalar(ipv[:, :, 0], posc[:], 2.0, 2.0,
                                        op0=OP.mult, op1=OP.subtract)
                nc.vector.tensor_scalar(ipv[:, :, 1], posc[:], 2.0, 1.0,
                                        op0=OP.mult, op1=OP.subtract)

                # ---- compact via local_scatter ----
                OI = scr.tile([P, POOL], U16, tag="OI")
                nc.vector.tensor_add(OI[:], PI[:], segbase[:])
                OIc = scr.tile([P, CCAP], U16, tag="OIc")
                nc.gpsimd.local_scatter(OIc[:], OI[:], idxB[:],
                                        channels=P, num_elems=CCAP,
                                        num_idxs=POOL)
                Vc = scr.tile([P, CCAP], F32, tag="Vc")
                nc.gpsimd.local_scatter(Vc.bitcast(U16)[:], Pv.bitcast(U16)[:],
                                        idxpair[:], channels=P,
                                        num_elems=2 * CCAP, num_idxs=2 * POOL)

                # ---- exact top-136 rounds on compacted array ----
                vsel = scr.tile([P, SEL], F32, tag="vsel")
                psel = scr.tile([P, SEL], U16, tag="psel")
                for r in range(ROUNDS):
                    nc.vector.max(out=vsel[:, r * 8:(r + 1) * 8], in_=Vc[:])
                    nc.vector.max_index(psel[:, r * 8:(r + 1) * 8],
                                        vsel[:, r * 8:(r + 1) * 8], Vc[:])
                    if r < ROUNDS - 1:
                        nc.vector.match_replace(Vc[:], vsel[:, r * 8:(r + 1) * 8],
                                                Vc[:], NEG)

                # ---- invert permutation: orig index per output slot ----
                invp = scr.tile([P, CCAP], U16, tag="invp")
                nc.gpsimd.local_scatter(invp[:], iota1u[:, :SEL],
                                        psel.bitcast(I16)[:],
                                        channels=P, num_elems=CCAP,
                                        num_idxs=SEL)
                invpf = scr.tile([P, CCAP], F32, tag="invpf")
                nc.vector.tensor_copy(invpf[:], invp[:])
                ipm1 = scr.tile([P, CCAP], I16, tag="ipm1")
                nc.vector.tensor_scalar(ipm1[:], invpf[:], 1.0, None,
                                        op0=OP.subtract)
                selo = scr.tile([P, SEL], U16, tag="selo")
                nc.gpsimd.local_scatter(selo[:], OIc[:], ipm1[:],
                                        channels=P, num_elems=SEL,
                                        num_idxs=CCAP)

                # ---- self removal (by index value) ----
                selof = scr.tile([P, SEL], F32, tag="selof")
                nc.vector.tensor_copy(selof[:], selo[:])
                sm = scr.tile([P, SEL], F32, tag="sm")
                nc.vector.tensor_scalar(sm[:], selof[:], selfidx_sb[:, t:t + 1],
                                        None, op0=OP.is_equal)
                sc = scr.tile([P, SEL], F32, tag="sc")
                nc.vector.tensor_tensor_scan(sc[:], sm[:], zeros[:, :SEL], 0.0,
                                             op0=OP.add, op1=OP.add)
                slotf = scr.tile([P, SEL], F32, tag="slotf")
                nc.vector.tensor_sub(slotf[:], iota0f[:], sc[:])
                vld = scr.tile([P, SEL], F32, tag="vld")
                nc.vector.tensor_scalar(vld[:], slotf[:], float(KOUT), None,
                                        op0=OP.is_lt)
                nsm = scr.tile([P, SEL], F32, tag="nsm")
                nc.vector.tensor_scalar(nsm[:], sm[:], -1.0, 1.0,
                                        op0=OP.mult, op1=OP.add)
                keep = scr.tile([P, SEL], F32, tag="keep")
                nc.vector.tensor_mul(keep[:], vld[:], nsm[:])
                slot1 = scr.tile([P, SEL], F32, tag="slot1")
                nc.vector.tensor_scalar(slot1[:], slotf[:], 1.0, None, op0=OP.add)
                posk = scr.tile([P, SEL], F32, tag="posk")
                nc.vector.tensor_mul(posk[:], slot1[:], keep[:])
                scidx = scr.tile([P, SEL], I16, tag="scidx")
                nc.vector.tensor_scalar(scidx[:], posk[:], 1.0, None,
                                        op0=OP.subtract)
                scpair = scr.tile([P, 2 * SEL], I16, tag="scpair")
                scv = scpair.rearrange("p (a b) -> p a b", b=2)
                nc.vector.tensor_scalar(scv[:, :, 0], posk[:], 2.0, 2.0,
                                        op0=OP.mult, op1=OP.subtract)
                nc.vector.tensor_scalar(scv[:, :, 1], posk[:], 2.0, 1.0,
                                        op0=OP.mult, op1=OP.subtract)

                oidx16 = opool.tile([P, KOUT], U16, tag="oidx16")
                nc.gpsimd.local_scatter(oidx16[:], selo[:], scidx[:],
                                        channels=P, num_elems=KOUT,
                                        num_idxs=SEL)
                vout = opool.tile([P, KOUT], F32, tag="vout")
                nc.gpsimd.local_scatter(vout.bitcast(U16)[:], vsel.bitcast(U16)[:],
                                        scpair[:], channels=P,
                                        num_elems=2 * KOUT, num_idxs=2 * SEL)

                # ---- outputs for this tile ----
                oidx32 = opool.tile([P, KOUT], I32, tag="oidx32")
                nc.vector.tensor_copy(oidx32[:], oidx16[:])
                nc.sync.dma_start(out=oidx[t * P:(t + 1) * P, :], in_=oidx32[:])
                d2t = opool.tile([P, KOUT], F32, tag="d2t")
                nc.vector.tensor_scalar(d2t[:], vout[:], -1.0, 0.0,
                                        op0=OP.mult, op1=OP.max)
                dist = opool.tile([P, KOUT], F32, tag="dist")
                nc.scalar.activation(dist[:], d2t[:], AF.Sqrt,
                                     bias=epscol[:, 0:1], scale=1.0)
                nc.sync.dma_start(out=odist[t * P:(t + 1) * P, :], in_=dist[:])
                junk = opool.tile([P, KOUT], F32, tag="junk")
                nc.vector.tensor_scalar(junk[:], dist[:], RADIUS, 0.0,
                                        op0=OP.is_lt, op1=OP.add,
                                        accum_out=cnts[:, t:t + 1])

            # ---- density, mean (AllReduce), adaptive k, mask ----
            cnt1 = cpool.tile([P, NT], F32)
            nc.vector.tensor_scalar(cnt1[:], cnts[:], 1.0, None, op0=OP.add)
            dens = cpool.tile([P, NT], F32)
            nc.vector.tensor_scalar(dens[:], cnt1[:], INVD, None,
                                    op0=OP.mult)
            # sum the integer-valued counts (exact in f32, order-independent)
            colsum = cpool.tile([P, NT], F32)
            nc.gpsimd.partition_all_reduce(colsum[:], cnt1[:], channels=P,
                                           reduce_op=bass.bass_isa.ReduceOp.add)
            tot = cpool.tile([P, 1], F32)
            nc.vector.tensor_reduce(tot[:], colsum[:], axis=mybir.AxisListType.X,
                                    op=OP.add)
            payload = cpool.tile([1, 2], F32)
            nc.vector.tensor_scalar(payload[:], bsel_sb[:], tot[0:1, 0:1], None,
                                    op0=OP.mult)
            ccin = dpool.tile([1, 2], F32)
            ccout = dpool.tile([1, 2], F32, addr_space="Shared")
            nc.sync.dma_start(out=ccin[:], in_=payload[:])
            if sim_single_core:
                nc.sync.dma_start(out=ccout[:], in_=ccin[:])
            else:
                nc.gpsimd.collective_compute(
                    "AllReduce", OP.add,
                    replica_groups=[list(range(8))],
                    ins=[ccin.opt()], outs=[ccout.opt()],
                )
            totals = cpool.tile([1, 2], F32)
            nc.sync.dma_start(out=totals[:], in_=ccout[:])
            myt = cpool.tile([1, 2], F32)
            nc.vector.tensor_mul(myt[:], totals[:], bsel_sb[:])
            tot2 = cpool.tile([1, 1], F32)
            nc.vector.tensor_reduce(tot2[:], myt[:], axis=mybir.AxisListType.X,
                                    op=OP.add)
            sdens = cpool.tile([1, 1], F32)
            nc.vector.tensor_scalar(sdens[:], tot2[:], INVD, None,
                                    op0=OP.mult)
            meansc = cpool.tile([1, 1], F32)
            nc.vector.tensor_scalar(meansc[:], sdens[:], 1.0 / N, None,
                                    op0=OP.mult)
            meanb = cpool.tile([P, 1], F32)
            nc.gpsimd.partition_broadcast(meanb[:], meansc[:])
            gsig = cpool.tile([1, 1], F32)
            nc.scalar.activation(gsig[:], gam_sb[:], AF.Sigmoid, bias=0.0,
                                 scale=1.0)
            gb = cpool.tile([P, 1], F32)
            nc.gpsimd.partition_broadcast(gb[:], gsig[:])

            deps = cpool.tile([P, NT], F32)
            nc.vector.tensor_scalar(deps[:], dens[:], EPS, None, op0=OP.add)
            recdep = cpool.tile([P, NT], F32)
            nc.vector.reciprocal(recdep[:], deps[:])
            ratio = cpool.tile([P, NT], F32)
            nc.vector.tensor_scalar(ratio[:], recdep[:], meanb[:, 0:1], None,
                                    op0=OP.mult)
            lnr = cpool.tile([P, NT], F32)
            nc.scalar.activation(lnr[:], ratio[:], AF.Ln, bias=0.0, scale=1.0)
            lng = cpool.tile([P, NT], F32)
            nc.vector.tensor_scalar(lng[:], lnr[:], gb[:, 0:1], None,
                                    op0=OP.mult)
            powr = cpool.tile([P, NT], F32)
            nc.scalar.activation(powr[:], lng[:], AF.Exp, bias=0.0, scale=1.0)
            kv1 = cpool.tile([P, NT], F32)
            nc.vector.tensor_scalar(kv1[:], powr[:], float(K_BASE), float(K_MIN),
                                    op0=OP.mult, op1=OP.max)
            kv = cpool.tile([P, NT], F32)
            nc.vector.tensor_scalar(kv[:], kv1[:], float(K_MAX), None,
                                    op0=OP.min)
            for t in range(NT):
                maskt = opool.tile([P, KOUT], U8, tag="maskt")
                nc.vector.tensor_scalar(maskt[:], iotah[:],
                                        kv[:, t:t + 1], None, op0=OP.is_le)
                nc.sync.dma_start(out=omask[t * P:(t + 1) * P, :], in_=maskt[:])

    nc.compile()
    return nc


_PROGRAM = None


def _get_program():
    global _PROGRAM
    if _PROGRAM is None:
        _PROGRAM = build_program()
    return _PROGRAM


def make_in_maps(coords, times, features, gamma_param):
    pts = np.concatenate(
        [np.asarray(coords, dtype=np.float32),
         np.asarray(times, dtype=np.float32)[..., None]], axis=-1)  # [B,N,3]
    x, y, tt = pts[..., 0], pts[..., 1], pts[..., 2]
    sq = ((x * x + y * y) + tt * tt).astype(np.float32)             # [B,N]
    gam = np.asarray(gamma_param, dtype=np.float32).reshape(1, 1)

    in_maps = []
    for c in range(8):
        b, r = c // 4, c % 4
        q0 = r * RPC
        ptsmT = np.ascontiguousarray(pts[b].T)                      # [3, N]
        in_maps.append({
            "ptsm": ptsmT,
            "ptss": np.ascontiguousarray(ptsmT[:, q0:q0 + RPC]),
            "sqq": np.ascontiguousarray(sq[b, q0:q0 + RPC].reshape(NT, P).T),
            "sqc": np.ascontiguousarray(sq[b][None, :]),
            "selfidx": np.ascontiguousarray(
                (q0 + np.arange(RPC, dtype=np.float32)).reshape(NT, P).T),
            "bsel": np.eye(2, dtype=np.float32)[b][None, :],
            "gamma_param": gam.copy(),
        })
    return in_maps


def assemble(results):
    idx = np.empty((B, N, KOUT), np.int32)
    msk = np.empty((B, N, KOUT), bool)
    dst = np.empty((B, N, KOUT), np.float32)
    for c in range(8):
        b, r = c // 4, c % 4
        q0 = r * RPC
        idx[b, q0:q0 + RPC] = results[c]["oidx"]
        msk[b, q0:q0 + RPC] = results[c]["omask"].astype(bool)
        dst[b, q0:q0 + RPC] = results[c]["odist"]
    return idx, msk, dst


def kernel(coords, times, features, gamma_param):
    nc = _get_program()
    in_maps = make_in_maps(coords, times, features, gamma_param)
    res = run_bass_kernel_spmd(nc, in_maps, core_ids=list(range(8)))
    return assemble(res.results)


# revision 19
# speedup vs baseline: 33.8837x; 33.8837x over previous
"""AdaptiveKNN Trainium2 kernel (8 NeuronCores, SPMD).

Sharding: data-parallel over batch B=2 across pairs-of-4 cores; within a
batch, the N=8192 query rows are row-sharded 4 ways (2048 rows/core).
Each core computes its [2048, 8192] distance block, per-row top-128
neighbors (sorted, with indices), the radius count, and (after a tiny
AllReduce for the batch mean density) the adaptive-k mask.

Distance values are computed with the exact same f32 rounding structure
as the reference (fp32 PE matmul for the dot product, single-rounding
broadcast add for sq_i+sq_j, fused 2*g - ssum with exact *2) so selected
values/order match a neuron-run reference bitwise.

Selection per 128-query tile:
  1) per-segment (S=128 -> 64 segs) top-8 values+local indices (DVE max8/max_index)
  2) cheap per-row threshold tau = 34th largest of the seg-4th-bests
     (guarantees >= 136 elements above tau)
  3) compact all elements >= tau (plus filler) to C=352 slots via
     rank-from-prefix-scan + GPSIMD local_scatter (16-bit pair trick for f32)
  4) exact 17x8 max8/max_index/match_replace rounds on the compacted array
  5) invert the selection permutation with two more local_scatters to get
     original indices; remove the self-match by index value; emit top-128.
"""

import os
import sys

sys.path.insert(0, "/opt/trn_rl_repo")

import numpy as np

import concourse.bass as bass
import concourse.bacc as bacc
import concourse.mybir as mybir
from concourse.tile import TileContext
from concourse.bass_utils import run_bass_kernel_spmd

F32 = mybir.dt.float32
U16 = mybir.dt.uint16
I16 = mybir.dt.int16
I32 = mybir.dt.int32
U8 = mybir.dt.uint8
AF = mybir.ActivationFunctionType
OP = mybir.AluOpType

B, N, D = 2, 8192, 64
K_BASE, K_MIN, K_MAX = 32, 8, 128
RADIUS, EPS = 0.05, 1e-8
KOUT = 128

P = 128            # partitions / queries per tile
RPC = 2048         # rows per core
NT = RPC // P      # row tiles per core (16)
NBLK = N // 512    # 512-wide candidate blocks per tile (16)
SEG = 64           # segment size for first-level top-8
NSEG = N // SEG    # 128
POOL = NSEG * 8    # 1024
CCAP = 352         # compaction capacity (empirical max 319 + margin)
ROUNDS = 17        # 17*8 = 136 extracted (incl self)
SEL = ROUNDS * 8   # 136
NEG = -3.0e38
D32 = float(np.float32(N * RADIUS**3 + EPS))  # density denominator, f32
INVD = float(np.float32(1.0 / np.float64(np.float32(D32))))  # XLA-folded 1/D


def build_program(sim_single_core=False):
    nc = bacc.Bacc("TRN2", target_bir_lowering=False,
                   num_devices=1 if sim_single_core else 8)

    ptsm = nc.declare_dram_parameter("ptsm", [3, N], F32, isOutput=False)
    ptss = nc.declare_dram_parameter("ptss", [3, RPC], F32, isOutput=False)
    sqq = nc.declare_dram_parameter("sqq", [P, NT], F32, isOutput=False)
    sqc = nc.declare_dram_parameter("sqc", [1, N], F32, isOutput=False)
    selfidx = nc.declare_dram_parameter("selfidx", [P, NT], F32, isOutput=False)
    bsel = nc.declare_dram_parameter("bsel", [1, 2], F32, isOutput=False)
    gam = nc.declare_dram_parameter("gamma_param", [1, 1], F32, isOutput=False)
    oidx = nc.declare_dram_parameter("oidx", [RPC, KOUT], I32, isOutput=True)
    odist = nc.declare_dram_parameter("odist", [RPC, KOUT], F32, isOutput=True)
    omask = nc.declare_dram_parameter("omask", [RPC, KOUT], U8, isOutput=True)

    with TileContext(nc) as tc:
        with (
            tc.tile_pool(name="const", bufs=1) as cpool,
            tc.tile_pool(name="sblk", bufs=3) as spool,
            tc.tile_pool(name="blk", bufs=2) as bpool,
            tc.tile_pool(name="poolp", bufs=2) as ppool,
            tc.tile_pool(name="scratch", bufs=1) as scr,
            tc.tile_pool(name="outs", bufs=2) as opool,
            tc.tile_pool(name="psum", bufs=6, space="PSUM") as psum,
            tc.tile_pool(name="dram", bufs=1, space="DRAM") as dpool,
        ):
            # ---- persistent constants / inputs ----
            ptsm_sb = cpool.tile([3, N], F32)
            ptss_sb = cpool.tile([3, RPC], F32)
            sqq_sb = cpool.tile([P, NT], F32)
            sqc_sb = cpool.tile([1, N], F32)
            selfidx_sb = cpool.tile([P, NT], F32)
            bsel_sb = cpool.tile([1, 2], F32)
            gam_sb = cpool.tile([1, 1], F32)
            nc.sync.dma_start(out=ptsm_sb[:], in_=ptsm[:])
            nc.sync.dma_start(out=ptss_sb[:], in_=ptss[:])
            nc.sync.dma_start(out=sqq_sb[:], in_=sqq[:])
            nc.sync.dma_start(out=sqc_sb[:], in_=sqc[:])
            nc.sync.dma_start(out=selfidx_sb[:], in_=selfidx[:])
            nc.sync.dma_start(out=bsel_sb[:], in_=bsel[:])
            nc.sync.dma_start(out=gam_sb[:], in_=gam[:])

            sqcb = cpool.tile([P, N], F32)
            nc.gpsimd.partition_broadcast(sqcb[:], sqc_sb[:])

            # constants built on device
            segbase = cpool.tile([P, POOL], U16)       # (q>>3)*128
            nc.gpsimd.iota(segbase[:], pattern=[[SEG, NSEG], [0, 8]], base=0,
                           channel_multiplier=0)
            iota1u = cpool.tile([P, POOL], U16)        # 1..512
            nc.gpsimd.iota(iota1u[:], pattern=[[1, POOL]], base=1,
                           channel_multiplier=0)
            iota1f = cpool.tile([P, POOL], F32)
            nc.vector.tensor_copy(iota1f[:], iota1u[:])
            iota0f = cpool.tile([P, SEL], F32)         # 0..135
            nc.vector.tensor_scalar(iota0f[:], iota1f[:, :SEL], 1.0, None,
                                    op0=OP.subtract)
            zeros = cpool.tile([P, POOL], F32)
            nc.vector.memset(zeros[:], 0.0)
            epscol = cpool.tile([P, 1], F32)
            nc.vector.memset(epscol[:], EPS)
            # c + 0.5 grid: neuron's f32->int32 cast rounds to nearest, so
            # mask[c] = c < round(k) == (c + 0.5 <= k) (ties ~never hit)
            iotah = cpool.tile([P, KOUT], F32)
            nc.vector.tensor_scalar(iotah[:], iota1f[:, :KOUT], 0.5, None,
                                    op0=OP.subtract)

            cnts = cpool.tile([P, NT], F32)            # radius counts per row

            # ---- per row-tile main loop ----
            for t in range(NT):
                Pv = ppool.tile([P, POOL], F32, tag="Pv")
                PI = ppool.tile([P, POOL], U16, tag="PI")

                for j in range(NBLK):
                    g_ps = psum.tile([P, 512], F32, tag="g_ps")
                    nc.tensor.matmul(
                        g_ps[:],
                        ptss_sb[:, t * P:(t + 1) * P],
                        ptsm_sb[:, j * 512:(j + 1) * 512],
                        start=True, stop=True,
                    )
                    ssum = bpool.tile([P, 512], F32, tag="ssum")
                    nc.scalar.activation(ssum[:], sqcb[:, j * 512:(j + 1) * 512],
                                         AF.Identity, bias=sqq_sb[:, t:t + 1],
                                         scale=1.0)
                    sb = spool.tile([P, 512], F32, tag="sb")
                    nc.vector.scalar_tensor_tensor(
                        sb[:], g_ps[:], 2.0, ssum[:],
                        op0=OP.mult, op1=OP.subtract)
                    for sg in range(8):
                        seg = 8 * j + sg
                        sl = sb[:, sg * SEG:(sg + 1) * SEG]
                        nc.vector.max(out=Pv[:, seg * 8:(seg + 1) * 8], in_=sl)
                        nc.vector.max_index(PI[:, seg * 8:(seg + 1) * 8],
                                            Pv[:, seg * 8:(seg + 1) * 8], sl)

                # ---- tau = 46th largest of the seg 3rd-bests ----
                P3c = Pv.rearrange("p (s k) -> p s k", k=8)[:, :, 2]  # [P, NSEG]
                w4 = scr.tile([P, NSEG], F32, tag="w4")
                m8 = scr.tile([P, 8], F32, tag="m8")
                nc.vector.max(out=m8[:], in_=P3c)
                nc.vector.match_replace(w4[:], m8[:], P3c, NEG)
                for r in range(4):
                    nc.vector.max(out=m8[:], in_=w4[:])
                    nc.vector.match_replace(w4[:], m8[:], w4[:], NEG)
                m8b = scr.tile([P, 8], F32, tag="m8b")
                nc.vector.max(out=m8b[:], in_=w4[:])   # ranks 41..48
                tau = m8b[:, 5:6]                       # 46th largest

                # ---- compaction ranks ----
                msk = scr.tile([P, POOL], F32, tag="msk")
                nc.vector.tensor_scalar(msk[:], Pv[:], tau, None, op0=OP.is_ge)
                rank1 = scr.tile([P, POOL], F32, tag="rank1")
                nc.vector.tensor_tensor_scan(rank1[:], msk[:], zeros[:], 0.0,
                                             op0=OP.add, op1=OP.add)
                rank1u = scr.tile([P, POOL], F32, tag="wC")
                nc.gpsimd.tensor_sub(rank1u[:], iota1f[:], rank1[:])
                posu = scr.tile([P, POOL], F32, tag="wD")
                nc.vector.tensor_scalar(posu[:], rank1u[:], rank1[:, POOL - 1:POOL],
                                        None, op0=OP.add)
                dru = scr.tile([P, POOL], F32, tag="wE")
                nc.gpsimd.tensor_sub(dru[:], rank1[:], posu[:])
                drm = scr.tile([P, POOL], F32, tag="wC")
                nc.gpsimd.tensor_mul(drm[:], msk[:], dru[:])
                pos1 = scr.tile([P, POOL], F32, tag="rank1")
                nc.gpsimd.tensor_add(pos1[:], posu[:], drm[:])
                mask2 = scr.tile([P, POOL], F32, tag="msk")
                nc.vector.tensor_scalar(mask2[:], pos1[:], float(CCAP), None,
                                        op0=OP.is_le)
                posc = scr.tile([P, POOL], F32, tag="wE")
                nc.vector.tensor_mul(posc[:], pos1[:], mask2[:])
                idxB = scr.tile([P, POOL], I16, tag="idxB")
                nc.vector.tensor_scalar(idxB[:], posc[:], 1.0, None,
                                        op0=OP.subtract)
                idxpair = scr.tile([P, 2 * POOL], I16, tag="idxpair")
                ipv = idxpair.rearrange("p (a b) -> p a b", b=2)
                nc.vector.tensor_scalar(ipv[:, :, 0], posc[:], 2.0, 2.0,
                                        op0=OP.mult, op1=OP.subtract)
                nc.vector.tensor_scalar(ipv[:, :, 1], posc[:], 2.0, 1.0,
                                        op0=OP.mult, op1=OP.subtract)

                # ---- compact via local_scatter ----
                OI = scr.tile([P, POOL], U16, tag="OI")
                nc.vector.tensor_add(OI[:], PI[:], segbase[:])
                OIc = scr.tile([P, CCAP], U16, tag="OIc")
                nc.gpsimd.local_scatter(OIc[:], OI[:], idxB[:],
                                        channels=P, num_elems=CCAP,
                                        num_idxs=POOL)
                Vc = scr.tile([P, CCAP], F32, tag="Vc")
                nc.gpsimd.local_scatter(Vc.bitcast(U16)[:], Pv.bitcast(U16)[:],
                                        idxpair[:], channels=P,
                                        num_elems=2 * CCAP, num_idxs=2 * POOL)

                # ---- exact top-136 rounds on compacted array ----
                vsel = scr.tile([P, SEL], F32, tag="vsel")
                psel = scr.tile([P, SEL], U16, tag="psel")
                for r in range(ROUNDS):
                    nc.vector.max(out=vsel[:, r * 8:(r + 1) * 8], in_=Vc[:])
                    nc.vector.max_index(psel[:, r * 8:(r + 1) * 8],
                                        vsel[:, r * 8:(r + 1) * 8], Vc[:])
                    if r < ROUNDS - 1:
                        nc.vector.match_replace(Vc[:], vsel[:, r * 8:(r + 1) * 8],
                                                Vc[:], NEG)

                # ---- invert permutation: orig index per output slot ----
                invp = scr.tile([P, CCAP], U16, tag="invp")
                nc.gpsimd.local_scatter(invp[:], iota1u[:, :SEL],
                                        psel.bitcast(I16)[:],
                                        channels=P, num_elems=CCAP,
                                        num_idxs=SEL)
                invpf = scr.tile([P, CCAP], F32, tag="invpf")
                nc.vector.tensor_copy(invpf[:], invp[:])
                ipm1 = scr.tile([P, CCAP], I16, tag="ipm1")
                nc.vector.tensor_scalar(ipm1[:], invpf[:], 1.0, None,
                                        op0=OP.subtract)
                selo = scr.tile([P, SEL], U16, tag="selo")
                nc.gpsimd.local_scatter(selo[:], OIc[:], ipm1[:],
                                        channels=P, num_elems=SEL,
                                        num_idxs=CCAP)

                # ---- self removal (by index value) ----
                selof = scr.tile([P, SEL], F32, tag="selof")
                nc.vector.tensor_copy(selof[:], selo[:])
                sm = scr.tile([P, SEL], F32, tag="sm")
                nc.vector.tensor_scalar(sm[:], selof[:], selfidx_sb[:, t:t + 1],
                                        None, op0=OP.is_equal)
                sc = scr.tile([P, SEL], F32, tag="sc")
                nc.vector.tensor_tensor_scan(sc[:], sm[:], zeros[:, :SEL], 0.0,
                                             op0=OP.add, op1=OP.add)
                slotf = scr.tile([P, SEL], F32, tag="slotf")
                nc.vector.tensor_sub(slotf[:], iota0f[:], sc[:])
                vld = scr.tile([P, SEL], F32, tag="vld")
                nc.vector.tensor_scalar(vld[:], slotf[:], float(KOUT), None,
                                        op0=OP.is_lt)
                nsm = scr.tile([P, SEL], F32, tag="nsm")
                nc.vector.tensor_scalar(nsm[:], sm[:], -1.0, 1.0,
                                        op0=OP.mult, op1=OP.add)
                keep = scr.tile([P, SEL], F32, tag="keep")
                nc.vector.tensor_mul(keep[:], vld[:], nsm[:])
                slot1 = scr.tile([P, SEL], F32, tag="slot1")
                nc.vector.tensor_scalar(slot1[:], slotf[:], 1.0, None, op0=OP.add)
                posk = scr.tile([P, SEL], F32, tag="posk")
                nc.vector.tensor_mul(posk[:], slot1[:], keep[:])
                scidx = scr.tile([P, SEL], I16, tag="scidx")
                nc.vector.tensor_scalar(scidx[:], posk[:], 1.0, None,
                                        op0=OP.subtract)
                scpair = scr.tile([P, 2 * SEL], I16, tag="scpair")
                scv = scpair.rearrange("p (a b) -> p a b", b=2)
                nc.vector.tensor_scalar(scv[:, :, 0], posk[:], 2.0, 2.0,
                                        op0=OP.mult, op1=OP.subtract)
                nc.vector.tensor_scalar(scv[:, :, 1], posk[:], 2.0, 1.0,
                                        op0=OP.mult, op1=OP.subtract)

                oidx16 = opool.tile([P, KOUT], U16, tag="oidx16")
                nc.gpsimd.local_scatter(oidx16[:], selo[:], scidx[:],
                                        channels=P, num_elems=KOUT,
                                        num_idxs=SEL)
                vout = opool.tile([P, KOUT], F32, tag="vout")
                nc.gpsimd.local_scatter(vout.bitcast(U16)[:], vsel.bitcast(U16)[:],
                                        scpair[:], channels=P,
                                        num_elems=2 * KOUT, num_idxs=2 * SEL)

                # ---- outputs for this tile ----
                oidx32 = opool.tile([P, KOUT], I32, tag="oidx32")
                nc.vector.tensor_copy(oidx32[:], oidx16[:])
                nc.sync.dma_start(out=oidx[t * P:(t + 1) * P, :], in_=oidx32[:])
                d2t = opool.tile([P, KOUT], F32, tag="d2t")
                nc.vector.tensor_scalar(d2t[:], vout[:], -1.0, 0.0,
                                        op0=OP.mult, op1=OP.max)
                dist = opool.tile([P, KOUT], F32, tag="dist")
                nc.scalar.activation(dist[:], d2t[:], AF.Sqrt,
                                     bias=epscol[:, 0:1], scale=1.0)
                nc.sync.dma_start(out=odist[t * P:(t + 1) * P, :], in_=dist[:])
                junk = opool.tile([P, KOUT], F32, tag="junk")
                nc.vector.tensor_scalar(junk[:], dist[:], RADIUS, 0.0,
                                        op0=OP.is_lt, op1=OP.add,
                                        accum_out=cnts[:, t:t + 1])

            # ---- density, mean (AllReduce), adaptive k, mask ----
            cnt1 = cpool.tile([P, NT], F32)
            nc.vector.tensor_scalar(cnt1[:], cnts[:], 1.0, None, op0=OP.add)
            dens = cpool.tile([P, NT], F32)
            nc.vector.tensor_scalar(dens[:], cnt1[:], INVD, None,
                                    op0=OP.mult)
            # sum the integer-valued counts (exact in f32, order-independent)
            colsum = cpool.tile([P, NT], F32)
            nc.gpsimd.partition_all_reduce(colsum[:], cnt1[:], channels=P,
                                           reduce_op=bass.bass_isa.ReduceOp.add)
            tot = cpool.tile([P, 1], F32)
            nc.vector.tensor_reduce(tot[:], colsum[:], axis=mybir.AxisListType.X,
                                    op=OP.add)
            payload = cpool.tile([1, 2], F32)
            nc.vector.tensor_scalar(payload[:], bsel_sb[:], tot[0:1, 0:1], None,
                                    op0=OP.mult)
            ccin = dpool.tile([1, 2], F32)
            ccout = dpool.tile([1, 2], F32, addr_space="Shared")
            nc.sync.dma_start(out=ccin[:], in_=payload[:])
            if sim_single_core:
                nc.sync.dma_start(out=ccout[:], in_=ccin[:])
            else:
                nc.gpsimd.collective_compute(
                    "AllReduce", OP.add,
                    replica_groups=[list(range(8))],
                    ins=[ccin.opt()], outs=[ccout.opt()],
                )
            totals = cpool.tile([1, 2], F32)
            nc.sync.dma_start(out=totals[:], in_=ccout[:])
            myt = cpool.tile([1, 2], F32)
            nc.vector.tensor_mul(myt[:], totals[:], bsel_sb[:])
            tot2 = cpool.tile([1, 1], F32)
            nc.vector.tensor_reduce(tot2[:], myt[:], axis=mybir.AxisListType.X,
                                    op=OP.add)
            sdens = cpool.tile([1, 1], F32)
            nc.vector.tensor_scalar(sdens[:], tot2[:], INVD, None,
                                    op0=OP.mult)
            meansc = cpool.tile([1, 1], F32)
            nc.vector.tensor_scalar(meansc[:], sdens[:], 1.0 / N, None,
                                    op0=OP.mult)
            meanb = cpool.tile([P, 1], F32)
            nc.gpsimd.partition_broadcast(meanb[:], meansc[:])
            gsig = cpool.tile([1, 1], F32)
            nc.scalar.activation(gsig[:], gam_sb[:], AF.Sigmoid, bias=0.0,
                                 scale=1.0)
            gb = cpool.tile([P, 1], F32)
            nc.gpsimd.partition_broadcast(gb[:], gsig[:])

            deps = cpool.tile([P, NT], F32)
            nc.vector.tensor_scalar(deps[:], dens[:], EPS, None, op0=OP.add)
            recdep = cpool.tile([P, NT], F32)
            nc.vector.reciprocal(recdep[:], deps[:])
            ratio = cpool.tile([P, NT], F32)
            nc.vector.tensor_scalar(ratio[:], recdep[:], meanb[:, 0:1], None,
                                    op0=OP.mult)
            lnr = cpool.tile([P, NT], F32)
            nc.scalar.activation(lnr[:], ratio[:], AF.Ln, bias=0.0, scale=1.0)
            lng = cpool.tile([P, NT], F32)
            nc.vector.tensor_scalar(lng[:], lnr[:], gb[:, 0:1], None,
                                    op0=OP.mult)
            powr = cpool.tile([P, NT], F32)
            nc.scalar.activation(powr[:], lng[:], AF.Exp, bias=0.0, scale=1.0)
            kv1 = cpool.tile([P, NT], F32)
            nc.vector.tensor_scalar(kv1[:], powr[:], float(K_BASE), float(K_MIN),
                                    op0=OP.mult, op1=OP.max)
            kv = cpool.tile([P, NT], F32)
            nc.vector.tensor_scalar(kv[:], kv1[:], float(K_MAX), None,
                                    op0=OP.min)
            for t in range(NT):
                maskt = opool.tile([P, KOUT], U8, tag="maskt")
                nc.vector.tensor_scalar(maskt[:], iotah[:],
                                        kv[:, t:t + 1], None, op0=OP.is_le)
                nc.sync.dma_start(out=omask[t * P:(t + 1) * P, :], in_=maskt[:])

    nc.compile()
    return nc


_PROGRAM = None


def _get_program():
    global _PROGRAM
    if _PROGRAM is None:
        _PROGRAM = build_program()
    return _PROGRAM


def make_in_maps(coords, times, features, gamma_param):
    pts = np.concatenate(
        [np.asarray(coords, dtype=np.float32),
         np.asarray(times, dtype=np.float32)[..., None]], axis=-1)  # [B,N,3]
    x, y, tt = pts[..., 0], pts[..., 1], pts[..., 2]
    sq = ((x * x + y * y) + tt * tt).astype(np.float32)             # [B,N]
    gam = np.asarray(gamma_param, dtype=np.float32).reshape(1, 1)

    in_maps = []
    for c in range(8):
        b, r = c // 4, c % 4
        q0 = r * RPC
        ptsmT = np.ascontiguousarray(pts[b].T)                      # [3, N]
        in_maps.append({
            "ptsm": ptsmT,
            "ptss": np.ascontiguousarray(ptsmT[:, q0:q0 + RPC]),
            "sqq": np.ascontiguousarray(sq[b, q0:q0 + RPC].reshape(NT, P).T),
            "sqc": np.ascontiguousarray(sq[b][None, :]),
            "selfidx": np.ascontiguousarray(
                (q0 + np.arange(RPC, dtype=np.float32)).reshape(NT, P).T),
            "bsel": np.eye(2, dtype=np.float32)[b][None, :],
            "gamma_param": gam.copy(),
        })
    return in_maps


def assemble(results):
    idx = np.empty((B, N, KOUT), np.int32)
    msk = np.empty((B, N, KOUT), bool)
    dst = np.empty((B, N, KOUT), np.float32)
    for c in range(8):
        b, r = c // 4, c % 4
        q0 = r * RPC
        idx[b, q0:q0 + RPC] = results[c]["oidx"]
        msk[b, q0:q0 + RPC] = results[c]["omask"].astype(bool)
        dst[b, q0:q0 + RPC] = results[c]["odist"]
    return idx, msk, dst


def kernel(coords, times, features, gamma_param):
    nc = _get_program()
    in_maps = make_in_maps(coords, times, features, gamma_param)
    res = run_bass_kernel_spmd(nc, in_maps, core_ids=list(range(8)))
    return assemble(res.results)
